# revision 4
# baseline (speedup 1.0000x reference)
"""Trainium2 Bass kernel for hierarchical 1D attention (HAttention1D).

Sharding: 8 cores = 4 batches x 2 sequence halves. Each core computes its
half's full pipeline: qkv projection, 9-level block-attention pyramid,
hierarchical combine, output projection. The only cross-half coupling is the
coarsest level (16 positions), supplied via a host-computed 256-mean-pool of
the other half's x (linearity of the projection makes this exact).

All matmuls bf16 with fp32 PSUM accumulation. The pair-flip of k/v blocks at
coarse levels is absorbed into constant mask patterns (block-diagonal vs
pair-swapped) added to the raw scores before exp; junk entries of the
all-pairs 128x128 score tile get -1e30 and vanish. Softmax row-max runs in
q-orientation; A^T for the A@V matmul comes from the DVE 32x32 stream
transpose, exact because the valid-block pattern is 32-block symmetric.
"""

import sys

sys.path.insert(0, "/opt/trn_rl_repo")

import functools
import numpy as np
import ml_dtypes

import concourse.bass as bass
import concourse.mybir as mybir
import concourse.tile as tile
from concourse import bacc
from concourse.bass_utils import run_bass_kernel_spmd

BF16 = mybir.dt.bfloat16
F32 = mybir.dt.float32
AOP = mybir.AluOpType
AFT = mybir.ActivationFunctionType
AXX = mybir.AxisListType

HEADS = 16
DH = 64
BSZ = 16
N = 4096                      # per-core half length
NEXT = 4224                   # + 16 xpool, 16 xpool*256, 96 zero pad
MS = [4096, 2048, 1024, 512, 256, 128, 64, 32, 16]   # level sizes l=0..8
PYR_OFF = {}                  # col offsets of levels 1..8 in qT/kT tiles
_o = NEXT
for _l in range(1, 9):
    PYR_OFF[_l] = _o
    _o += MS[_l]
LTOT = _o + 16                # + 16 zero pad after l8 (for padded l8 stationary)
SLOT_BASE = {0: 0, 1: 32, 2: 48, 3: 56, 4: 60, 5: 62, 6: 63, 7: 64, 8: 65}
NSLOT = 66
PCHUNKS = [(i * 512, 512) for i in range(8)] + [(4096, 128)]   # proj moving chunks


def build_nc():
    nc = bacc.Bacc(None, target_bir_lowering=False)

    xt = nc.dram_tensor("xt", [1024, NEXT], BF16, kind="ExternalInput")
    wqk = nc.dram_tensor("wqk", [1024, HEADS, 128], BF16, kind="ExternalInput")
    wv = nc.dram_tensor("wv", [1024, 1024], BF16, kind="ExternalInput")
    wout = nc.dram_tensor("wout", [1024, 1024], BF16, kind="ExternalInput")
    bb = nc.dram_tensor("bb", [128, 1024], F32, kind="ExternalInput")
    mdiag = nc.dram_tensor("mdiag", [128, 128], F32, kind="ExternalInput")
    mflip = nc.dram_tensor("mflip", [128, 128], F32, kind="ExternalInput")
    pool = nc.dram_tensor("pool", [128, 64], BF16, kind="ExternalInput")
    out = nc.dram_tensor("out", [N, 1024], F32, kind="ExternalOutput")

    with tile.TileContext(nc) as tc:
        with (
            tc.tile_pool(name="cst", bufs=1) as cst,
            tc.tile_pool(name="dram", bufs=1, space="DRAM") as dram,
            tc.tile_pool(name="work", bufs=2) as work,
            tc.tile_pool(name="ps_proj", bufs=2, space="PSUM") as ps_proj,
            tc.tile_pool(name="ps_s", bufs=2, space="PSUM") as ps_s,
            tc.tile_pool(name="ps_y", bufs=2, space="PSUM") as ps_y,
            tc.tile_pool(name="ps_vp", bufs=2, space="PSUM") as ps_vp,
        ):
            vnat_dram = dram.tile([NEXT, 1024], BF16)
            ynt_dram = dram.tile([1024, N], BF16)

            mdiag_sb = cst.tile([128, 128], F32)
            mflip_sb = cst.tile([128, 128], F32)
            pool_sb = cst.tile([128, 64], BF16)
            bb_sb = cst.tile([128, 1024], F32)
            nc.sync.dma_start(mdiag_sb[:], mdiag[:])
            nc.sync.dma_start(mflip_sb[:], mflip[:])
            nc.sync.dma_start(pool_sb[:], pool[:])
            nc.sync.dma_start(bb_sb[:], bb[:])

            with tc.tile_pool(name="p12", bufs=1) as p12:
                xt_sb = p12.tile([128, 8, NEXT], BF16)
                nc.sync.dma_start(
                    xt_sb[:], xt.rearrange("(kt p) n -> p kt n", p=128)
                )

                # ---------- P1: v projection (all heads), pos-major ----------
                import contextlib
                wvp = contextlib.ExitStack()
                wvpool = wvp.enter_context(tc.tile_pool(name="wvp", bufs=1))
                wv_sb = wvpool.tile([128, 8, 1024], BF16)
                nc.sync.dma_start(
                    wv_sb[:], wv.rearrange("(kt p) n -> p kt n", p=128)
                )
                for pt in range(NEXT // 128):
                    for c in range(2):
                        vps = ps_proj.tile([128, 512], F32, tag="psproj")
                        for kt in range(8):
                            nc.tensor.matmul(
                                vps[:],
                                xt_sb[:, kt, pt * 128:(pt + 1) * 128],
                                wv_sb[:, kt, c * 512:(c + 1) * 512],
                                start=(kt == 0), stop=(kt == 7),
                            )
                        vstage = work.tile([128, 512], BF16, tag="vstage")
                        nc.vector.tensor_copy(vstage[:], vps[:])
                        nc.sync.dma_start(
                            vnat_dram[pt * 128:(pt + 1) * 128,
                                      c * 512:(c + 1) * 512],
                            vstage[:],
                        )

                wvp.close()

                # ---------- P2: per-head projection + attention ----------
                p2stack = contextlib.ExitStack()
                accp = p2stack.enter_context(tc.tile_pool(name="accp", bufs=1))
                divp = p2stack.enter_context(tc.tile_pool(name="divp", bufs=1))
                for h in range(HEADS):
                    wqk_h = work.tile([128, 8, 128], BF16, tag="wqkh")
                    nc.sync.dma_start(
                        wqk_h[:],
                        wqk[:, h, :].rearrange("(kt p) c -> p kt c", p=128),
                    )
                    qT = accp.tile([64, LTOT], BF16, tag="qT")
                    kT = accp.tile([64, LTOT], BF16, tag="kT")
                    # zero the l8 stationary pad region (cols l8..l8+32)
                    nc.vector.memset(qT[:, PYR_OFF[8] + 16:PYR_OFF[8] + 32], 0.0)

                    for (coff, clen) in PCHUNKS:
                        qkps = ps_proj.tile([128, 512], F32, tag="psproj")
                        for kt in range(8):
                            nc.tensor.matmul(
                                qkps[:, :clen],
                                wqk_h[:, kt, :],
                                xt_sb[:, kt, coff:coff + clen],
                                start=(kt == 0), stop=(kt == 7),
                            )
                        nc.vector.tensor_copy(qT[:, coff:coff + clen],
                                              qkps[0:64, :clen])
                        nc.vector.tensor_copy(kT[:, coff:coff + clen],
                                              qkps[64:128, :clen])

                    # q/k sum-pool pyramids (free-dim pairwise adds)
                    for l in range(1, 9):
                        m = MS[l]
                        for t in (qT, kT):
                            src = (t[:, 0:4096] if l == 1
                                   else t[:, PYR_OFF[l - 1]:PYR_OFF[l - 1] + MS[l - 1]])
                            sv = src.rearrange("p (m two) -> p m two", two=2)
                            nc.vector.tensor_tensor(
                                t[:, PYR_OFF[l]:PYR_OFF[l] + m],
                                sv[:, :, 0], sv[:, :, 1], AOP.add,
                            )

                    # vext slots: [128, slot, 128] = [v | ones]
                    vext = accp.tile([128, NSLOT, 128], BF16, tag="vext")
                    nc.vector.memset(vext[:, :, 64:128], 1.0)
                    nc.vector.memset(vext[:, 63:66, 0:64], 0.0)
                    nc.sync.dma_start(
                        vext[:, 0:32, 0:64],
                        vnat_dram[0:4096, h * 64:(h + 1) * 64].rearrange(
                            "(g p) d -> p g d", p=128),
                    )
                    nc.sync.dma_start(
                        vext[0:16, 65, 0:64],
                        vnat_dram[4112:4128, h * 64:(h + 1) * 64],
                    )
                    # v pyramids via PE pooling
                    for l in range(1, 8):
                        m = MS[l]
                        if m >= 128:
                            for c in range(m // 128):
                                for half in range(2):
                                    pps = ps_vp.tile([64, 64], F32, tag="psvp")
                                    nc.tensor.matmul(
                                        pps[:],
                                        pool_sb[:, 0:64],
                                        vext[:, SLOT_BASE[l - 1] + 2 * c + half, 0:64],
                                        start=True, stop=True,
                                    )
                                    nc.vector.tensor_copy(
                                        vext[half * 64:(half + 1) * 64,
                                             SLOT_BASE[l] + c, 0:64],
                                        pps[:],
                                    )
                        else:
                            msrc = MS[l - 1]
                            pps = ps_vp.tile([64, 64], F32, tag="psvp")
                            nc.tensor.matmul(
                                pps[0:m, :],
                                pool_sb[0:msrc, 0:m],
                                vext[0:msrc, SLOT_BASE[l - 1], 0:64],
                                start=True, stop=True,
                            )
                            nc.vector.tensor_copy(
                                vext[0:m, SLOT_BASE[l], 0:64], pps[0:m, :])

                    # attention slots, coarse -> fine
                    yaccs = {}
                    for l in range(8, -1, -1):
                        m = min(MS[l], 128) if l < 8 else 32
                        nslots = max(MS[l] // 128, 1) if l < 8 else 1
                        mvalid = MS[l] if l < 8 else 16
                        yacc = accp.tile([128, MS[l] if l < 8 else 16],
                                         F32, tag=f"yacc{l}")
                        yaccs[l] = yacc
                        scale = (0.25 ** l) / 8.0 if l < 8 else (0.5 ** 8) / 8.0
                        if l == 0 or l == 8:
                            msk = mdiag_sb
                        else:
                            msk = mflip_sb
                        for g in range(nslots):
                            if l < 8:
                                qoff = (128 * g if l == 0
                                        else PYR_OFF[l] + 128 * g)
                                koff = qoff
                            else:
                                qoff = PYR_OFF[8]
                                koff = 4096
                            sps = ps_s.tile([128, 128], F32, tag="pss")
                            nc.tensor.matmul(
                                sps[0:m, 0:m],
                                qT[:, qoff:qoff + m],
                                kT[:, koff:koff + m],
                                start=True, stop=True,
                            )
                            sm = work.tile([128, 128], F32, tag="sm")
                            nc.vector.tensor_tensor(
                                sm[0:m, 0:m], sps[0:m, 0:m],
                                msk[0:m, 0:m], AOP.add)
                            negb = work.tile([128, 1], F32, tag="negb")
                            nc.vector.reduce_max(
                                negb[0:m, :], sm[0:m, 0:m],
                                axis=AXX.X, negate=True)
                            nsc = work.tile([128, 1], F32, tag="nsc")
                            nc.vector.tensor_scalar_mul(
                                nsc[0:m, :], negb[0:m, :], scale)
                            ab = work.tile([128, 128], BF16, tag="ab")
                            nc.scalar.activation(
                                ab[0:m, 0:m], sm[0:m, 0:m], AFT.Exp,
                                bias=nsc[0:m, :], scale=scale)
                            atb = work.tile([128, 128], BF16, tag="atb")
                            nc.vector.transpose(atb[0:m, 0:m], ab[0:m, 0:m])
                            yps = ps_y.tile([128, 128], F32, tag="psy")
                            slot = SLOT_BASE[l] + g
                            nc.tensor.matmul(
                                yps[:, 0:m],
                                vext[0:m, slot, :],
                                atb[0:m, 0:m],
                                start=True, stop=True,
                            )
                            # telescope
                            mv = min(mvalid, 128)
                            if l == 8:
                                nc.vector.tensor_copy(yacc[:, 0:16],
                                                      yps[:, 0:16])
                            else:
                                half = mv // 2
                                prev = yaccs[l + 1]
                                pv = prev[:, half * g:half * g + half]
                                nc.vector.tensor_tensor(
                                    yacc[:, mv * g:mv * g + mv].rearrange(
                                        "p (a b) -> p a b", b=2),
                                    yps[:, 0:mv].rearrange(
                                        "p (a b) -> p a b", b=2),
                                    pv[:, :, None].to_broadcast(
                                        (128, half, 2)),
                                    AOP.add,
                                )

                    # divide + store ynT (chunked)
                    y0 = yaccs[0]
                    ynt = divp.tile([64, 4096], BF16, tag="ynt")
                    for dc in range(4):
                        sl = slice(dc * 1024, (dc + 1) * 1024)
                        arow = divp.tile([64, 1024], F32, tag="arow")
                        nc.vector.tensor_copy(arow[:], y0[64:128, sl])
                        rcp = divp.tile([64, 1024], F32, tag="rcp")
                        nc.vector.reciprocal(rcp[:], arow[:])
                        nc.vector.tensor_tensor(ynt[:, sl], y0[0:64, sl],
                                                rcp[:], AOP.mult)
                    nc.sync.dma_start(
                        ynt_dram[h * 64:(h + 1) * 64, :], ynt[:])

                p2stack.close()
            # ---------- P3: output projection ----------
            with tc.tile_pool(name="p3", bufs=1) as p3:
                ynt_sb = p3.tile([128, 8, N], BF16)
                nc.sync.dma_start(
                    ynt_sb[:], ynt_dram[:].rearrange("(kt p) n -> p kt n", p=128))
                wout_sb = p3.tile([128, 8, 1024], BF16)
                nc.sync.dma_start(
                    wout_sb[:], wout.rearrange("(kt p) n -> p kt n", p=128))
                for pt in range(32):
                    ostage = work.tile([128, 1024], F32, tag="ostage")
                    for c in range(2):
                        ops = ps_proj.tile([128, 512], F32, tag="psproj")
                        for kt in range(8):
                            nc.tensor.matmul(
                                ops[:],
                                ynt_sb[:, kt, pt * 128:(pt + 1) * 128],
                                wout_sb[:, kt, c * 512:(c + 1) * 512],
                                start=(kt == 0), stop=(kt == 7),
                            )
                        nc.vector.tensor_tensor(
                            ostage[:, c * 512:(c + 1) * 512], ops[:],
                            bb_sb[:, c * 512:(c + 1) * 512], AOP.add)
                    nc.sync.dma_start(
                        out[pt * 128:(pt + 1) * 128, :], ostage[:])

    nc.compile()
    return nc


@functools.lru_cache(maxsize=1)
def _cached_nc():
    return build_nc()


def _host_inputs(x, w_qkv, w_out, b_out):
    bf = ml_dtypes.bfloat16
    wq, wk, wv = w_qkv[:, 0:1024], w_qkv[:, 1024:2048], w_qkv[:, 2048:3072]
    wqk = np.empty((1024, HEADS, 128), dtype=np.float32)
    for h in range(HEADS):
        wqk[:, h, 0:64] = wq[:, h * 64:(h + 1) * 64]
        wqk[:, h, 64:128] = wk[:, h * 64:(h + 1) * 64]
    wqk = wqk.astype(bf)
    wvb = wv.astype(bf)
    woutb = w_out.astype(bf)
    bb = np.tile(b_out.astype(np.float32)[None, :], (128, 1))

    mdiag = np.full((128, 128), -1e30, np.float32)
    mflip = np.full((128, 128), -1e30, np.float32)
    for b in range(8):
        mdiag[b * 16:(b + 1) * 16, b * 16:(b + 1) * 16] = 0.0
        p = b ^ 1
        mflip[b * 16:(b + 1) * 16, p * 16:(p + 1) * 16] = 0.0
    pool = np.zeros((128, 64), np.float32)
    for i in range(128):
        pool[i, i // 2] = 1.0
    pool = pool.astype(bf)

    shared = dict(wqk=wqk, wv=wvb, wout=woutb, bb=bb,
                  mdiag=mdiag, mflip=mflip, pool=pool)

    in_maps = []
    for core in range(8):
        b, s = core // 2, core % 2
        xh = x[b, s * N:(s + 1) * N, :]
        xo = x[b, (1 - s) * N:(2 - s) * N, :]
        xpool = xo.reshape(16, 256, 1024).mean(axis=1)
        xte = np.zeros((1024, NEXT), np.float32)
        xte[:, 0:N] = xh.T
        xte[:, N:N + 16] = xpool.T
        xte[:, N + 16:N + 32] = xpool.T * 256.0
        in_maps.append(dict(shared, xt=xte.astype(bf)))
    return in_maps


def kernel(x, w_qkv, w_out, b_out):
    nc = _cached_nc()
    in_maps = _host_inputs(np.asarray(x, np.float32), np.asarray(w_qkv, np.float32),
                           np.asarray(w_out, np.float32), np.asarray(b_out, np.float32))
    res = run_bass_kernel_spmd(nc, in_maps, core_ids=list(range(8)))
    out = np.empty((4, 8192, 1024), np.float32)
    for core in range(8):
        b, s = core // 2, core % 2
        out[b, s * N:(s + 1) * N, :] = res.results[core]["out"]
    return out
